# revision 1
# baseline (speedup 1.0000x reference)
"""Trainium2 Bass kernel for causal self-attention (GQA + q/k RMSNorm + RoPE).

Sharding: tensor-parallel over heads across 8 NeuronCores. Core c computes
q-heads {2c, 2c+1} and their shared kv head c//2 end-to-end (projections,
attention, and the partial output projection out_c = Y_c @ wc[rows_c]); the
host sums the 8 partial outputs.

All matmuls run as float32r (TF32 speed, fp32 PSUM accumulation; measured
~6e-4 max rel err end-to-end); inputs are TF32-rounded on the host.
Projections compute [Q0|Q1|K|V] fused per 128-token block (lhsT = x^T tile,
rhs = concatenated weights), RMSNorm uses a DVE square+reduce plus a Newton
rsqrt (keeps ACT on a single Exp table set), RoPE is elementwise on
stride-2 pairs, and q/k are PE-transposed to [d, token] for the attention
matmuls S^T = K^T.T @ Q^T, l = ones.T @ P, Y^T = V.T @ P. exp(scale*S + causal
log-mask) runs on ACT straight out of PSUM; softmax needs no max-subtraction
because rmsnorm bounds |scores| <= sqrt(HD)*max|norm_w|^2.
"""

import numpy as np

B, T, C = 2, 2048, 2048
NH, NKV, HD = 16, 4, 128
NCORES = 8
HPC = NH // NCORES  # q heads per core = 2
EPS = 1e-5
ROPE_BASE = 10000.0
SCALE = 1.0 / float(np.sqrt(HD))
NEG = -100.0  # additive log-mask for causally-forbidden entries
KT = C // 128  # 16 contraction tiles for the projections
QTILE = 512
STILE = 128
NQT = T // QTILE  # 4 q-tiles per batch
NTT = T // QTILE  # token tiles per batch in projection phase

_CACHE: dict = {}


def _round_tf32(a: np.ndarray) -> np.ndarray:
    u = np.ascontiguousarray(a, dtype=np.float32).view(np.uint32).copy()
    u += 0xFFF + ((u >> 13) & 1)
    u &= np.uint32(0xFFFFE000)
    return u.view(np.float32)


CUT = ""  # timing-only knobs: "attn", "out", "proj" reduce work in that phase


def _build(reps: int = 1, phases: str = "pao"):
    """phases: subset of 'p' (projections), 'a' (attention), 'o' (out-proj)."""
    import concourse.tile as tile
    from concourse import bacc, mybir

    F32R = mybir.dt.float32r
    F32 = mybir.dt.float32
    AF = mybir.ActivationFunctionType

    nc = bacc.Bacc("TRN2", target_bir_lowering=False, debug=False)

    def din(name, shape, dt_=F32R):
        return nc.dram_tensor(name, shape, dt_, kind="ExternalInput").ap()

    xT_d = din("xT", [C, B * T])
    wqkv_d = din("wqkv", [C, 4 * HD])
    wc_d = din("wc", [HPC * HD, C])
    cosp_d = din("cosp", [128, (T // 128) * 64], F32)
    sinp_d = din("sinp", [128, (T // 128) * 64], F32)
    lmask_d = din("lmask", [128, 896], F32)
    consts_d = din("consts", [128, 516])
    out_d = nc.dram_tensor("out", [B * T, C], F32, kind="ExternalOutput").ap()

    xT_re = xT_d.rearrange("(kc p) t -> p kc t", p=128)  # [128,16,4096]
    wqkv_re = wqkv_d.rearrange("(kc p) m -> p kc m", p=128)  # [128,16,512]
    wc_re = wc_d.rearrange("(dp p) c -> p dp c", p=128)  # [128,2,2048]

    with tile.TileContext(nc) as tc:
        import contextlib

        ctx = contextlib.ExitStack()
        with ctx:
            const = ctx.enter_context(tc.tile_pool(name="const", bufs=1))
            qkv = ctx.enter_context(tc.tile_pool(name="qkv", bufs=1))
            ypool = ctx.enter_context(tc.tile_pool(name="y", bufs=1))
            xpool = ctx.enter_context(tc.tile_pool(name="x", bufs=4))
            work = ctx.enter_context(tc.tile_pool(name="wk", bufs=2))
            ptp = ctx.enter_context(tc.tile_pool(name="pt", bufs=3))
            rows = ctx.enter_context(tc.tile_pool(name="rows", bufs=2))
            outst = ctx.enter_context(tc.tile_pool(name="outst", bufs=10))
            psA = ctx.enter_context(tc.tile_pool(name="psA", bufs=4, space="PSUM"))
            psB = ctx.enter_context(tc.tile_pool(name="psB", bufs=2, space="PSUM"))
            psPV = ctx.enter_context(tc.tile_pool(name="psPV", bufs=1, space="PSUM"))
            psLS = ctx.enter_context(tc.tile_pool(name="psLS", bufs=1, space="PSUM"))

            # ---- resident weights/tables ----
            wqkv_sb = const.tile([128, KT, 4 * HD], F32R)
            wc_sb = const.tile([128, HPC, C], F32R)
            cosp = const.tile([128, (T // 128) * 64], F32)
            sinp = const.tile([128, (T // 128) * 64], F32)
            lmask = const.tile([128, 896], F32)
            consts = const.tile([128, 516], F32R)
            nc.sync.dma_start(wqkv_sb[:], wqkv_re)
            nc.sync.dma_start(wc_sb[:], wc_re)
            nc.sync.dma_start(cosp[:], cosp_d)
            nc.sync.dma_start(sinp[:], sinp_d)
            nc.sync.dma_start(lmask[:], lmask_d)
            nc.sync.dma_start(consts[:], consts_d)
            ident = consts[:, 0:128]
            ones_c = consts[:, 128:129]
            ones_r = consts[0:1, 129:257]
            qw_row = consts[0:1, 257:385]
            kw_row = consts[0:1, 385:513]
            eps_sb = consts[0:1, 513:514].bitcast(F32)
            epsc = const.tile([128, 1], F32)
            nc.vector.memset(epsc[:], EPS)
            # materialize W2q/W2k [128,128] = ones (x) norm_weight once
            w2q = const.tile([128, HD], F32)
            w2k = const.tile([128, HD], F32)
            _wp = psB.tile([128, HD], F32, tag="b", name="wp_q")
            nc.tensor.matmul(_wp[:], ones_r, qw_row, start=True, stop=True)
            nc.scalar.copy(w2q[:], _wp[:])
            _wp2 = psB.tile([128, HD], F32, tag="b", name="wp_k")
            nc.tensor.matmul(_wp2[:], ones_r, kw_row, start=True, stop=True)
            nc.scalar.copy(w2k[:], _wp2[:])

            MAGIC = 0x5F3759DF

            def rsqrt3(m3, y3, t3):
                """y3 = 1/sqrt(m3) elementwise on [128,3] via 2 Newton steps."""
                I32 = mybir.dt.int32
                nc.vector.tensor_scalar(
                    t3.bitcast(I32), m3.bitcast(I32), 1, None,
                    op0=mybir.AluOpType.logical_shift_right,
                )
                nc.vector.tensor_scalar(
                    y3.bitcast(I32), t3.bitcast(I32), -1, MAGIC,
                    op0=mybir.AluOpType.mult, op1=mybir.AluOpType.add,
                )
                for _ in range(2):
                    nc.vector.tensor_mul(t3, y3, y3)
                    nc.vector.tensor_mul(t3, t3, m3)
                    nc.vector.tensor_scalar(
                        t3, t3, -0.5, op0=mybir.AluOpType.mult,
                        scalar2=1.5, op1=mybir.AluOpType.add,
                    )
                    nc.vector.tensor_mul(y3, y3, t3)

            def norm_rope_block(pos, w2, tkb, dst_col, mcol, ycol):
                """Apply norm scale + RoPE to one [128 tok, 128 d] block (pos,
                SBUF f32), transpose to [d, tok], store to dst_col (F32R)."""
                qn = work.tile([128, HD], F32, tag="qn")
                nc.vector.scalar_tensor_tensor(
                    qn[:], pos[:], ycol, w2[:],
                    op0=mybir.AluOpType.mult, op1=mybir.AluOpType.mult,
                )
                # rope on interleaved pairs (stride-2 free slices)
                qv = qn[:].rearrange("p (d two) -> p two d", two=2)
                cs = cosp[:, tkb * 64 : (tkb + 1) * 64]
                sn = sinp[:, tkb * 64 : (tkb + 1) * 64]
                u1 = work.tile([128, 64], F32, tag="u1")
                u2 = work.tile([128, 64], F32, tag="u2")
                rp = work.tile([128, HD], F32R, tag="rp")
                rv = rp[:].rearrange("p (d two) -> p two d", two=2)
                nc.vector.tensor_mul(u1[:], qv[:, 0, :], cs)
                nc.vector.tensor_mul(u2[:], qv[:, 1, :], sn)
                nc.vector.tensor_sub(rv[:, 0, :], u1[:], u2[:])
                nc.vector.tensor_mul(u1[:], qv[:, 0, :], sn)
                nc.vector.tensor_mul(u2[:], qv[:, 1, :], cs)
                nc.vector.tensor_add(rv[:, 1, :], u1[:], u2[:])
                # transpose [tok, d] -> [d, tok]
                trp = psB.tile([128, HD], F32R, tag="b", name="tr_nr")
                nc.tensor.transpose(trp[:], rp[:], ident)
                nc.scalar.copy(dst_col, trp[:].bitcast(F32))

            def body():
                for b in range(B):
                    tb = b * T
                    qT = qkv.tile([128, HPC, T], F32R, tag="qT")
                    kT = qkv.tile([128, T], F32R, tag="kT")
                    vsb = qkv.tile([128, T // 128, 128], F32R, tag="v")
                    yT = ypool.tile([128, HPC, T], F32R, tag="yT")

                    # ---- projections: per 128-token block, one fused
                    # [Q0|Q1|K|V] accumulation (lhsT = xT block, rhs = wqkv) ----
                    for tkb in range(T // 128 if "p" in phases else 0):
                        tk0 = tb + tkb * 128
                        xt = xpool.tile([128, KT, 128], F32R, tag="xt")
                        for j4 in range(4):
                            nc.sync.dma_start(
                                xt[:, j4 * 4 : (j4 + 1) * 4, :],
                                xT_re[:, j4 * 4 : (j4 + 1) * 4, tk0 : tk0 + 128],
                            )
                        po = psA.tile([128, 4 * HD], F32, tag="a")
                        for kc in range(1 if CUT == "proj" else KT):
                            nc.tensor.matmul(
                                po[:], xt[:, kc, :], wqkv_sb[:, kc, :],
                                start=(kc == 0), stop=(kc == KT - 1 or CUT == "proj"),
                            )
                        ct = rows.tile([128, 9], F32, tag="cols", bufs=3)
                        dsts = [
                            qT[:, 0, tkb * 128 : (tkb + 1) * 128],
                            qT[:, 1, tkb * 128 : (tkb + 1) * 128],
                            kT[:, tkb * 128 : (tkb + 1) * 128],
                        ]
                        w2s = [w2q, w2q, w2k]
                        poss = []
                        for si3 in range(3):
                            pos = work.tile([128, HD], F32, tag=f"pos{si3}", name=f"pos{si3}")
                            nc.scalar.copy(pos[:], po[:, si3 * HD : (si3 + 1) * HD])
                            nsc = work.tile([128, HD], F32, tag="nsc")
                            nc.vector.tensor_mul(nsc[:], pos[:], pos[:])
                            nc.vector.reduce_sum(
                                ct[:, si3 : si3 + 1], nsc[:],
                                axis=mybir.AxisListType.X,
                            )
                            poss.append(pos)
                        nc.scalar.copy(vsb[:, tkb, :], po[:, 3 * HD : 4 * HD])
                        nc.vector.tensor_scalar(
                            ct[:, 0:3], ct[:, 0:3], 1.0 / HD, EPS,
                            op0=mybir.AluOpType.mult, op1=mybir.AluOpType.add,
                        )
                        rsqrt3(ct[:, 0:3], ct[:, 3:6], ct[:, 6:9])
                        for si3 in range(3):
                            norm_rope_block(
                                poss[si3], w2s[si3], tkb, dsts[si3],
                                ct[:, si3 : si3 + 1], ct[:, 3 + si3 : 4 + si3],
                            )

                    # ---- attention per head ----
                    for h in range(HPC if "a" in phases else 0):
                        for qi in range(NQT):
                            q0 = qi * QTILE
                            n_s = 4 * qi + 4
                            ps_y = psPV.tile([128, QTILE], F32, tag="pv")
                            ps_l = psLS.tile([1, QTILE], F32, tag="ls")
                            for si in range(1 if CUT == "attn" else n_s):
                                ps_s = psB.tile([128, QTILE], F32, tag="b")
                                nc.tensor.matmul(
                                    ps_s[:],
                                    kT[:, si * 128 : (si + 1) * 128],
                                    qT[:, h, q0 : q0 + QTILE],
                                    start=True,
                                    stop=True,
                                )
                                pt = ptp.tile([128, QTILE], F32R, tag="pt")
                                j = si - 4 * qi
                                if j >= 0:
                                    sm = work.tile([128, QTILE], F32, tag="sm")
                                    nc.vector.scalar_tensor_tensor(
                                        sm[:],
                                        ps_s[:],
                                        SCALE,
                                        lmask[:, 384 - 128 * j : 896 - 128 * j],
                                        op0=mybir.AluOpType.mult,
                                        op1=mybir.AluOpType.add,
                                    )
                                    nc.scalar.activation(pt[:], sm[:], AF.Exp)
                                else:
                                    nc.scalar.activation(pt[:], ps_s[:], AF.Exp, scale=SCALE)
                                st = si == 0
                                sp = si == n_s - 1
                                nc.tensor.matmul(
                                    ps_l[:], ones_c, pt[:], start=st, stop=sp
                                )
                                nc.tensor.matmul(
                                    ps_y[:], vsb[:, si, :], pt[:], start=st, stop=sp
                                )
                            # normalize: yT = ps_y * (1/l) broadcast
                            rt = rows.tile([1, QTILE], F32, tag="rowsf")
                            rl = rt[0:1, :]
                            nc.vector.reciprocal(rl, ps_l[:])
                            rtr = rows.tile([1, QTILE], F32R, tag="rowsr")
                            rlr = rtr[0:1, :]
                            nc.vector.tensor_copy(rlr, rl)
                            bcp = psB.tile([128, QTILE], F32, tag="b")
                            nc.tensor.matmul(bcp[:], ones_r, rlr, start=True, stop=True)
                            bc = work.tile([128, QTILE], F32, tag="ybc")
                            nc.vector.tensor_copy(bc[:], bcp[:])
                            nc.vector.tensor_mul(
                                yT[:, h, q0 : q0 + QTILE], ps_y[:], bc[:]
                            )

                    # ---- output projection (partial over this core's heads) ----
                    for ti in range(T // 128 if "o" in phases else 0):
                        accs = [
                            psA.tile([128, QTILE], F32, tag="a", name=f"acc_o{ci}")
                            for ci in range(4)
                        ]
                        for h in range(HPC):
                            for ci in range(1 if CUT == "out" else 4):
                                nc.tensor.matmul(
                                    accs[ci][:],
                                    yT[:, h, ti * 128 : (ti + 1) * 128],
                                    wc_sb[:, h, ci * QTILE : (ci + 1) * QTILE],
                                    start=(h == 0),
                                    stop=(h == HPC - 1),
                                )
                        for ci in range(1 if CUT == "out" else 4):
                            ob = outst.tile([128, QTILE], F32, tag="ob")
                            nc.scalar.copy(ob[:], accs[ci][:])
                            nc.sync.dma_start(
                                out_d[
                                    tb + ti * 128 : tb + (ti + 1) * 128,
                                    ci * QTILE : (ci + 1) * QTILE,
                                ],
                                ob[:],
                            )

            if reps == 1:
                body()
            else:
                with tc.For_i(0, reps, 1):
                    body()

    nc.compile()
    return nc


def _host_inputs(x, wq, wk, wv, wc, q_norm_w, k_norm_w):
    """Build the 8 per-core input dicts."""
    x2 = np.ascontiguousarray(np.asarray(x, dtype=np.float32).reshape(B * T, C))
    xT = _round_tf32(np.ascontiguousarray(x2.T))

    pos = np.arange(T, dtype=np.float64)
    inv_freq = 1.0 / (ROPE_BASE ** (np.arange(0, HD, 2, dtype=np.float64) / HD))
    theta = pos[:, None] * inv_freq[None, :]  # [T, 64]
    cosv = np.cos(theta).astype(np.float32)  # [T, 64]
    sinv = np.sin(theta).astype(np.float32)
    # [128, ntk*64]: cosp[p, tkb*64+f] = cos((tkb*128+p) * invf[f])
    ntk = T // 128
    cosp = np.ascontiguousarray(
        cosv.reshape(ntk, 128, 64).transpose(1, 0, 2).reshape(128, ntk * 64)
    )
    sinp = np.ascontiguousarray(
        sinv.reshape(ntk, 128, 64).transpose(1, 0, 2).reshape(128, ntk * 64)
    )

    # lm[p, 384+u] = 0 if u >= p else NEG; diag tile j reads [:, 384-128j : 896-128j]
    p = np.arange(128)[:, None]
    u = np.arange(896)[None, :] - 384
    lm = np.where(u >= p, 0.0, NEG).astype(np.float32)



    wq = np.asarray(wq, dtype=np.float32)
    wk = np.asarray(wk, dtype=np.float32)
    wv = np.asarray(wv, dtype=np.float32)
    wc = np.asarray(wc, dtype=np.float32)
    qw = np.asarray(q_norm_w, dtype=np.float32)
    kw = np.asarray(k_norm_w, dtype=np.float32)
    consts = np.zeros((128, 516), dtype=np.float32)
    consts[:, 0:128] = np.eye(128, dtype=np.float32)
    consts[:, 128] = 1.0
    consts[0, 129:257] = 1.0
    consts[0, 257:385] = _round_tf32(qw)
    consts[0, 385:513] = _round_tf32(kw)
    consts[0, 513] = EPS

    in_maps = []
    for c in range(NCORES):
        h0 = HPC * c
        g = h0 // (NH // NKV)
        wqkv = np.concatenate(
            [
                wq[:, h0 * HD : (h0 + HPC) * HD],
                wk[:, g * HD : (g + 1) * HD],
                wv[:, g * HD : (g + 1) * HD],
            ],
            axis=1,
        )
        wrows = np.arange(h0 * HD, (h0 + HPC) * HD)
        in_maps.append(
            {
                "xT": xT,
                "wqkv": _round_tf32(wqkv),
                "wc": _round_tf32(wc[wrows, :]),
                "cosp": cosp,
                "sinp": sinp,
                "lmask": lm,
                "consts": consts,
            }
        )
    return in_maps


def kernel(x, wq, wk, wv, wc, q_norm_w, k_norm_w):
    from concourse.bass_utils import run_bass_kernel_spmd

    if "nc" not in _CACHE:
        _CACHE["nc"] = _build()
    nc = _CACHE["nc"]
    in_maps = _host_inputs(x, wq, wk, wv, wc, q_norm_w, k_norm_w)
    res = run_bass_kernel_spmd(nc, in_maps, core_ids=list(range(NCORES)))
    out = np.zeros((B * T, C), dtype=np.float32)
    for r in res.results:
        out += r["out"]
    return out.reshape(B, T, C)

